# revision 1
# baseline (speedup 1.0000x reference)
"""Trainium2 Bass kernel for EnhancedMultiHeadSelfAttention (dense transformer block).

Sharding: sequence-parallel over 8 cores. Each core owns L/8 = 256 query rows.
K/V for all 2048 tokens are computed on every core from fp8 x; everything else
(Q, scores, softmax, attn@V, out-proj, LN2, FFN) is own-rows only. No
collectives.

Key structure (all activations feature-major [feature, token]):
 - LN1 is never applied to x. Projections run on raw fp8 x with an extra
   K=1 DoubleRow "correction row": out += (-colsum(W))*mu_t + b*sigma_t.
   The per-token rstd then cancels in cosine normalization for Q/K; for V,
   both rstd and the (key-side) lcc softmax bias are folded into a per-key
   scale applied during V's PSUM->SBUF copy, with the softmax denominator
   coming from an extra V column holding sigma*rstd*e^lcc = e^lcc terms.
 - K is cosine-normalized in place (fp8) so the exp over scores has a
   CONSTANT scale and no bias: one Activation op covers both heads of a
   pair (512 elements), halving Act-engine overhead on the exp path.
 - Q/K/V/out-proj and attn@V run as fp8e4 DoubleRow matmuls (two 128-row
   K-tiles per instruction at 0.5 cycles/row). FFN runs in bf16 (fp8 would
   exceed the 2e-2 error budget through the 4096-wide contraction).
 - clip(scores,-10,10) never binds and softmax needs no max-subtraction.

Scale conventions (stored value = scale * true value):
  x8 = 16 x     w{q,k,v,o}8 = 64 w    proj PSUM = 1024 * true
  k8 = 4 k~ then (after normalize) 16 k-hat      q8 = 16 q-hat
  v8 = 4 e^b v~ (b = key lcc bias + ln rstd)     mu8 = 256 mu
  sd8 = 16 sigma    den col = 16 e^b sigma       attn8 = 16 attn
"""

import numpy as np
import ml_dtypes

import concourse.bass as bass
import concourse.tile as tile
from concourse import bacc, mybir
from concourse.bass_utils import run_bass_kernel_spmd

F32 = mybir.dt.float32
F32R = mybir.dt.float32r
BF16 = mybir.dt.bfloat16
F8 = mybir.dt.float8e4
NP_F8 = ml_dtypes.float8_e4m3
NP_BF16 = ml_dtypes.bfloat16

L = 2048          # sequence length
D = 1024          # model dim
H = 16            # heads
DH = 64           # head dim
FF = 4096         # ffn hidden
P = 128           # partitions
NCORES = 8
LQ = L // NCORES  # 256 own query rows per core
DC = D // P       # 8 d-model chunks
FC = FF // P      # 32 ffn chunks
KC = L // P       # 16 key chunks
NBLK = 4          # token blocks of 512
BLK = L // NBLK   # 512

# CoreSim doesn't implement Gelu; test_sim swaps this to Identity and checks
# against a gelu-less reference. Hardware always uses the real (erf) Gelu.
GELU_FUNC = mybir.ActivationFunctionType.Gelu

LN_EPS = 1e-5
SCALING = DH ** -0.5
LCC = 0.1
DR = mybir.MatmulPerfMode.DoubleRow

SX = 16.0     # x8 scale
SW = 64.0     # fp8 weight scale
PS = SX * SW  # = 1024, scale of projection PSUM results
SKV = 4.0     # k~/v~ scale
SK8 = 16.0    # normalized k-hat scale
SQ8 = 16.0    # q8 scale (unit vectors * 16)
SMU = 256.0   # mu8 scale
SSD = 16.0    # sd8 (sigma) scale
SA = 16.0     # attn8 scale


def emit(tc):
    nc = tc.nc
    AF = mybir.ActivationFunctionType

    xt = nc.dram_tensor("xt", [D, L], BF16, kind="ExternalInput").ap()
    xot = nc.dram_tensor("xot", [D, LQ], F32, kind="ExternalInput").ap()
    xotb = nc.dram_tensor("xotb", [D, LQ], BF16, kind="ExternalInput").ap()
    wq8 = nc.dram_tensor("wq8", [P, DC, D], F8, kind="ExternalInput").ap()
    wk8 = nc.dram_tensor("wk8", [P, DC, D], F8, kind="ExternalInput").ap()
    wv8 = nc.dram_tensor("wv8", [P, DC, D], F8, kind="ExternalInput").ap()
    wo8 = nc.dram_tensor("wo8", [P, DC, D], F8, kind="ExternalInput").ap()
    corq = nc.dram_tensor("corq", [1, 2, D], F8, kind="ExternalInput").ap()
    cork = nc.dram_tensor("cork", [1, 2, D], F8, kind="ExternalInput").ap()
    corv = nc.dram_tensor("corv", [1, 2, D], F8, kind="ExternalInput").ap()
    wf1h = nc.dram_tensor("wf1h", [P, FC, DC, P], BF16, kind="ExternalInput").ap()
    wf2h = nc.dram_tensor("wf2h", [P, FC, DC, P], BF16, kind="ExternalInput").ap()
    bo = nc.dram_tensor("bo", [P, DC], F32, kind="ExternalInput").ap()
    bf1 = nc.dram_tensor("bf1", [P, FC], F32, kind="ExternalInput").ap()
    bf2 = nc.dram_tensor("bf2", [P, DC], F32, kind="ExternalInput").ap()
    lccel = nc.dram_tensor("lccel", [P, KC], F32, kind="ExternalInput").ap()
    selr = nc.dram_tensor("selr", [P, P], F8, kind="ExternalInput").ap()
    selrb = nc.dram_tensor("selrb", [P, P], BF16, kind="ExternalInput").ap()
    selb = nc.dram_tensor("selb", [H, DC * P], F32R, kind="ExternalInput").ap()
    onesbd = nc.dram_tensor("onesbd", [P, 1], BF16, kind="ExternalInput").ap()
    ones1r = nc.dram_tensor("ones1r", [1, P], F32R, kind="ExternalInput").ap()
    c64r = nc.dram_tensor("c64r", [1, P], F32R, kind="ExternalInput").ap()
    onescl = nc.dram_tensor("onescl", [P, 1], F32R, kind="ExternalInput").ap()
    out_t = nc.dram_tensor("out_t", [D, LQ], F32, kind="ExternalOutput").ap()

    xt3 = xt.rearrange("(c p) t -> p c t", p=P)        # [128, 8, 2048]
    xot3 = xot.rearrange("(c p) t -> p c t", p=P)      # [128, 8, 256]
    out3 = out_t.rearrange("(c p) t -> p c t", p=P)    # [128, 8, 256]

    mm = nc.tensor.matmul

    # ---- persistent small constants -------------------------------------
    singles = tc.alloc_tile_pool(name="singles", bufs=1)
    ones_1x128 = singles.tile([1, P], F32R)  # K=1 broadcast lhsT (value 1)
    nc.scalar.dma_start(ones_1x128, ones1r)
    c64row = singles.tile([1, P], F32R)      # K=1 broadcast lhsT (value 64)
    nc.scalar.dma_start(c64row, c64r)
    ones_col = singles.tile([P, 1], F32R)    # K=128 -> M=1 reduction lhsT
    nc.scalar.dma_start(ones_col, onescl)
    onesb = singles.tile([P, 1], BF16)       # bf16 reduction lhsT
    nc.scalar.dma_start(onesb, onesbd)
    selr8 = singles.tile([P, DC, H], F8)
    nc.scalar.dma_start(selr8, selr.rearrange("p (m h) -> p m h", h=H))
    selrb_sb = singles.tile([P, DC, H], BF16)
    nc.scalar.dma_start(selrb_sb, selrb.rearrange("p (m h) -> p m h", h=H))
    selb_sb = singles.tile([H, DC, P], F32R)
    nc.scalar.dma_start(selb_sb, selb.rearrange("h (m p) -> h m p", p=P))
    corq_sb = singles.tile([1, 2, D], F8)
    nc.scalar.dma_start(corq_sb, corq)
    cork_sb = singles.tile([1, 2, D], F8)
    nc.scalar.dma_start(cork_sb, cork)
    corv_sb = singles.tile([1, 2, D], F8)
    nc.scalar.dma_start(corv_sb, corv)
    bo_sb = singles.tile([P, DC], F32)
    nc.scalar.dma_start(bo_sb, bo)
    bf1_sb = singles.tile([P, FC], F32)
    nc.scalar.dma_start(bf1_sb, bf1)
    bf2_sb = singles.tile([P, DC], F32)
    nc.scalar.dma_start(bf2_sb, bf2)
    lcce_sb = singles.tile([P, KC], F32)
    nc.scalar.dma_start(lcce_sb, lccel)
    eps_sb = singles.tile([P, 1], F32)
    nc.vector.memset(eps_sb, LN_EPS)
    # fp8 full weights (4 x 8KB/partition)
    wq_sb = singles.tile([P, DC, D], F8)
    nc.sync.dma_start(wq_sb, wq8)
    wk_sb = singles.tile([P, DC, D], F8)
    nc.sync.dma_start(wk_sb, wk8)
    wv_sb = singles.tile([P, DC, D], F8)
    wo_sb = singles.tile([P, DC, D], F8)
    # per-token stat tensors (filled during phases A/B)
    musd8 = singles.tile([1, 2, L], F8)       # [mu8 ; sd8] rows
    vscale = singles.tile([P, KC], F32)       # (SKV/PS) * rstd * e^lcc
    col8 = singles.tile([P, KC], F8)          # SSD * rstd * e^lcc (den col)
    reck = singles.tile([H, L], F32R)         # 4 / |k~| rows
    stat_r = singles.tile([P, NBLK, 2, 4], F32)   # [p, b, (sum,sumsq), kc%4]
    musd_r = singles.tile([P, NBLK, 2, 4], F8)    # repartitioned mu8/sd8

    scr_pool = tc.alloc_tile_pool(name="scr", bufs=1, space="DRAM")
    scr_st = scr_pool.tile([NBLK, 2, BLK], F32)    # stats rows -> repart
    scr_ms = scr_pool.tile([NBLK, 2, BLK], F8)     # mu/sd repart -> rows

    # ---- persistent activation tensors ----------------------------------
    outp = tc.alloc_tile_pool(name="outp", bufs=1)
    x2 = outp.tile([P, DC, LQ], F32R)
    attn8 = outp.tile([P, DC, LQ], F8)
    wf1s = tc.alloc_tile_pool(name="wf1s", bufs=9)
    wf2s = tc.alloc_tile_pool(name="wf2s", bufs=6)
    midp = tc.alloc_tile_pool(name="midp", bufs=1)
    k8 = midp.tile([P, DC, L], F8)
    VW = 80  # 64 values + 1 denominator + 15 pad (dual-fp8 M%16==0)
    v_sb = midp.tile([P, KC, H, VW], F8)
    q8 = midp.tile([P, DC, LQ], F8)
    x8p = tc.alloc_tile_pool(name="x8p", bufs=1)
    x8 = x8p.tile([P, DC, L], F8)

    # =====================================================================
    # Phase A: x -> fp8, per-token stats; Phase C: Q projection + normalize
    # (emitted together so the scheduler can overlap them)
    # =====================================================================
    with (
        tc.tile_pool(name="xblk", bufs=2) as xblk_pool,
        tc.tile_pool(name="xsqp", bufs=2) as xsq_pool,
        tc.tile_pool(name="stm", bufs=2) as stm_pool,
        tc.tile_pool(name="strow", bufs=1) as strow_pool,
        tc.tile_pool(name="qcp", bufs=1) as qc_pool,
        tc.tile_pool(name="qsqp", bufs=2) as qsq_pool,
        tc.tile_pool(name="ps_st", bufs=1, space="PSUM") as ps_st,
        tc.tile_pool(name="ps_q", bufs=2, space="PSUM") as ps_q,
        tc.tile_pool(name="ps_qn", bufs=1, space="PSUM") as ps_qn,
        tc.tile_pool(name="ps_qb", bufs=1, space="PSUM") as ps_qb,
    ):
        for b in range(NBLK):
            sl = slice(b * BLK, (b + 1) * BLK)
            xblk = xblk_pool.tile([P, DC, BLK], BF16, tag="xblk")
            nc.gpsimd.dma_start(xblk, xt3[:, :, sl])
            if b == NBLK - 1:
                # V/out-proj weights load after all x blocks are queued
                nc.gpsimd.dma_start(wv_sb, wv8)
                nc.gpsimd.dma_start(wo_sb, wo8)
            with nc.allow_low_precision(reason="fp8 pipeline"):
                nc.scalar.activation(x8[:, :, sl], xblk, func=AF.Copy,
                                     bias=0.0, scale=SX)
                xsqb = xsq_pool.tile([P, DC, BLK], BF16, tag="xsq")
                nc.vector.tensor_mul(xsqb, xblk, xblk)
            sums = ps_st.tile([1, BLK], F32, tag="sums")
            sumsq = ps_st.tile([1, BLK], F32, tag="sumsq")
            for c in range(DC):
                mm(sums, onesb, xblk[:, c, :], start=(c == 0),
                   stop=(c == DC - 1))
                mm(sumsq, onesb, xsqb[:, c, :], start=(c == 0),
                   stop=(c == DC - 1))
            # stage stat rows to SBUF, roundtrip via DRAM to [128, ...] layout
            statrow = strow_pool.tile([1, 2, BLK], F32, tag="strow")
            nc.scalar.copy(statrow[:, 0, :], sums)
            nc.scalar.copy(statrow[:, 1, :], sumsq)
            nc.sync.dma_start(scr_st[b:b + 1], statrow)
            nc.sync.dma_start(
                stat_r[:, b, :, :],
                scr_st[b].rearrange("j (q p) -> p j q", p=P))
            # per-token coefficient math in [128, 4] layout
            mu = stm_pool.tile([P, 4], F32, tag="mu")
            nc.vector.tensor_scalar_mul(mu, stat_r[:, b, 0, :], 1.0 / D)
            ex2 = stm_pool.tile([P, 4], F32, tag="ex2")
            nc.vector.tensor_scalar_mul(ex2, stat_r[:, b, 1, :], 1.0 / D)
            var = stm_pool.tile([P, 4], F32, tag="var")
            nc.vector.tensor_mul(var, mu, mu)
            nc.vector.tensor_sub(var, ex2, var)
            sd = stm_pool.tile([P, 4], F32, tag="sd")
            nc.scalar.activation(sd, var, func=AF.Sqrt, bias=eps_sb, scale=1.0)
            rstd = stm_pool.tile([P, 4], F32, tag="rstd")
            with nc.allow_low_precision(reason="coef"):
                nc.vector.reciprocal(rstd, sd)
            kcs = slice(b * 4, (b + 1) * 4)
            relcc = stm_pool.tile([P, 4], F32, tag="relcc")
            nc.vector.tensor_mul(relcc, rstd, lcce_sb[:, kcs])
            nc.vector.tensor_scalar_mul(vscale[:, kcs], relcc, SKV / PS)
            with nc.allow_low_precision(reason="fp8 pipeline"):
                nc.vector.tensor_scalar_mul(col8[:, kcs], relcc, SSD)
                nc.vector.tensor_scalar_mul(musd_r[:, b, 0, :], mu, SMU)
                nc.vector.tensor_scalar_mul(musd_r[:, b, 1, :], sd, SSD)
            nc.sync.dma_start(
                scr_ms[b].rearrange("j (q p) -> p j q", p=P),
                musd_r[:, b, :, :])
            for j in range(2):
                nc.sync.dma_start(
                    musd8[:, j, sl],
                    scr_ms[b, j].rearrange("(o t) -> o t", o=1))

        # ---- Phase C: Q (own tokens; stats recomputed from xot since the
        # shared program can't address its own slice of musd8) --------------
        xo_blk = qc_pool.tile([P, DC, LQ], BF16, name="xo_blk")
        nc.sync.dma_start(xo_blk, xotb.rearrange("(c p) t -> p c t", p=P))
        x8own = qc_pool.tile([P, DC, LQ], F8, name="x8own")
        with nc.allow_low_precision(reason="fp8 pipeline"):
            nc.scalar.activation(x8own, xo_blk, func=AF.Copy, bias=0.0,
                                 scale=SX)
        ps_os = ps_qn.tile([1, LQ], F32, tag="osum")
        for c in range(DC):
            mm(ps_os, onesb, xo_blk[:, c, :], start=(c == 0),
               stop=(c == DC - 1))
        osr = qc_pool.tile([1, 2, LQ], F32, name="osr")
        nc.vector.tensor_copy(osr[:, 0, :], ps_os)
        xsq_o = qc_pool.tile([P, DC, LQ], BF16, name="xsq_o")
        with nc.allow_low_precision(reason="bf16 pipeline"):
            nc.vector.tensor_mul(xsq_o, xo_blk, xo_blk)
        for c in range(DC):
            mm(ps_os, onesb, xsq_o[:, c, :], start=(c == 0),
               stop=(c == DC - 1))
        nc.vector.tensor_copy(osr[:, 1, :], ps_os)
        mu_o = qc_pool.tile([1, LQ], F32, name="mu_o")
        nc.vector.tensor_scalar_mul(mu_o, osr[:, 0, :], 1.0 / D)
        ex2_o = qc_pool.tile([1, LQ], F32, name="ex2_o")
        nc.vector.tensor_scalar_mul(ex2_o, osr[:, 1, :], 1.0 / D)
        var_o = qc_pool.tile([1, LQ], F32, name="var_o")
        nc.vector.tensor_mul(var_o, mu_o, mu_o)
        nc.vector.tensor_sub(var_o, ex2_o, var_o)
        sd_o = qc_pool.tile([1, LQ], F32, name="sd_o")
        nc.scalar.activation(sd_o, var_o, func=AF.Sqrt, bias=eps_sb[0:1, :],
                             scale=1.0)
        musd_own = qc_pool.tile([1, 2, LQ], F8, name="musd_own")
        with nc.allow_low_precision(reason="fp8 pipeline"):
            nc.vector.tensor_scalar_mul(musd_own[:, 0, :], mu_o, SMU)
            nc.vector.tensor_scalar_mul(musd_own[:, 1, :], sd_o, SSD)
        # Q DoubleRow projections + per-head cosine normalization
        qt_sb = qc_pool.tile([P, DC, LQ], BF16, name="qt_sb")
        nsq_q = ps_qn.tile([H, LQ], F32, tag="qn")
        qsq8 = None
        for m in range(DC):
            ps = ps_q.tile([P, LQ], F32, tag="qps")
            for i in range(4):
                mm(ps, wq_sb[:, 2 * i:2 * i + 2, m * P:(m + 1) * P],
                   x8own[:, 2 * i:2 * i + 2, :], start=(i == 0), stop=False,
                   perf_mode=DR)
            mm(ps, corq_sb[:, :, m * P:(m + 1) * P], musd_own,
               start=False, stop=True, perf_mode=DR)
            with nc.allow_low_precision(reason="bf16 pipeline"):
                nc.vector.tensor_scalar_mul(qt_sb[:, m, :], ps, SKV / PS)
            if m % 2 == 0:
                qsq8 = qsq_pool.tile([P, 2, LQ], F8, tag="qsq")
            nc.scalar.activation(qsq8[:, m % 2, :], ps, func=AF.Square,
                                 bias=0.0, scale=2.0 / PS)
            if m % 2 == 1:
                mm(nsq_q, selr8[:, m - 1:m + 1, :], qsq8,
                   start=(m == 1), stop=(m == DC - 1), perf_mode=DR)
        qs = qc_pool.tile([H, LQ], F32, name="qs")
        nc.scalar.activation(qs, nsq_q, func=AF.Sqrt, bias=0.0, scale=1.0)
        nc.vector.tensor_scalar_max(qs, qs, 1e-8)
        rec = qc_pool.tile([H, LQ], F32R, name="qrec")
        with nc.allow_low_precision(reason="coef"):
            nc.vector.reciprocal(rec, qs)
        nc.vector.tensor_scalar_mul(rec, rec, 2.0 * SQ8 / SKV)
        for m in range(DC):
            bc = ps_qb.tile([P, LQ], F32, tag="qbc")
            mm(bc, selb_sb[:, m, :], rec, start=True, stop=True)
            with nc.allow_low_precision(reason="fp8 pipeline"):
                nc.vector.tensor_mul(q8[:, m, :], qt_sb[:, m, :], bc)

    # sigma/e^lcc column of V (denominator source) + zero pad columns
    with nc.allow_low_precision(reason="fp8 pipeline"):
        nc.vector.memset(v_sb[:, :, :, DH + 1:VW], 0.0)
        nc.vector.tensor_copy(
            v_sb[:, :, :, DH], col8.unsqueeze(2).to_broadcast([P, KC, H]))

    # =====================================================================
    # Phase B: K and V projections (fp8 DR); K cosine-normalized in place
    # =====================================================================
    with (
        tc.tile_pool(name="ksqp", bufs=2) as ksq_pool,
        tc.tile_pool(name="nsqs", bufs=2) as nsqs_pool,
        tc.tile_pool(name="ps_k", bufs=2, space="PSUM") as ps_k,
        tc.tile_pool(name="ps_v", bufs=2, space="PSUM") as ps_v,
        tc.tile_pool(name="ps_n", bufs=2, space="PSUM") as ps_n,
    ):
        for b in range(NBLK):
            sl = slice(b * BLK, (b + 1) * BLK)
            ms = musd8[:, :, sl]
            for m in range(DC):
                ps = ps_k.tile([P, BLK], F32, tag="kps")
                for i in range(4):
                    mm(ps, wk_sb[:, 2 * i:2 * i + 2, m * P:(m + 1) * P],
                       x8[:, 2 * i:2 * i + 2, sl], start=(i == 0), stop=False,
                       perf_mode=DR)
                mm(ps, cork_sb[:, :, m * P:(m + 1) * P], ms,
                   start=False, stop=True, perf_mode=DR)
                with nc.allow_low_precision(reason="fp8 pipeline"):
                    if m % 2 == 0:
                        nc.vector.tensor_scalar_mul(k8[:, m, sl], ps, SKV / PS)
                    else:
                        nc.scalar.activation(k8[:, m, sl], ps, func=AF.Copy,
                                             bias=0.0, scale=SKV / PS)
            # V for this block's 4 token chunks (scaled per key by vscale)
            for t in range(b * 4, (b + 1) * 4):
                tsl = slice(t * P, (t + 1) * P)
                for g in range(2):
                    csl = slice(g * BLK, (g + 1) * BLK)
                    ps = ps_v.tile([P, BLK], F32, tag="vps")
                    for i in range(4):
                        mm(ps, x8[:, 2 * i:2 * i + 2, tsl],
                           wv_sb[:, 2 * i:2 * i + 2, csl],
                           start=(i == 0), stop=False, perf_mode=DR)
                    mm(ps, musd8[:, :, tsl], corv_sb[:, :, csl],
                       start=False, stop=True, perf_mode=DR)
                    ps_h = ps.rearrange("p (h d) -> p h d", d=DH)
                    dst = v_sb[:, t, g * DC:(g + 1) * DC, 0:DH]
                    with nc.allow_low_precision(reason="fp8 pipeline"):
                        if g == 0:
                            nc.vector.tensor_scalar_mul(
                                dst, ps_h, vscale[:, t:t + 1])
                        else:
                            nc.scalar.activation(dst, ps_h, func=AF.Copy,
                                                 bias=0.0,
                                                 scale=vscale[:, t:t + 1])
            # k norms: squares on Pool (bf16), per-head sums, then reck
            nsq = ps_n.tile([H, BLK], F32, tag="nsq")
            for m in range(DC):
                ksqb = ksq_pool.tile([P, BLK], BF16, tag="ksq")
                with nc.allow_low_precision(reason="bf16 pipeline"):
                    nc.gpsimd.tensor_mul(ksqb, k8[:, m, sl], k8[:, m, sl])
                mm(nsq, selrb_sb[:, m, :], ksqb, start=(m == 0),
                   stop=(m == DC - 1))
            nsq_sb = nsqs_pool.tile([H, BLK], F32, tag="nsqs")
            nc.scalar.activation(nsq_sb, nsq, func=AF.Sqrt, bias=0.0,
                                 scale=1.0)
            nc.vector.tensor_scalar_max(nsq_sb, nsq_sb, 1e-8)
            with nc.allow_low_precision(reason="coef"):
                nc.vector.reciprocal(reck[:, sl], nsq_sb)
            nc.vector.tensor_scalar_mul(reck[:, sl], reck[:, sl], SK8)

    x8p.release()

    # =====================================================================
    # Phases D/E/F (full query width): scores -> exp -> attn@V -> out-proj
    # -> LN2 -> FFN
    # =====================================================================
    EXPS = SCALING / (SQ8 * SK8)

    ffp = tc.alloc_tile_pool(name="ffp", bufs=1)
    h_t = ffp.tile([P, FC, LQ], BF16)
    normed2 = ffp.tile([P, DC, LQ], BF16)
    dep = tc.alloc_tile_pool(name="dep", bufs=1)
    xo2 = dep.tile([P, DC, LQ], F32)
    nc.sync.dma_start(xo2, xot3)

    with (
        tc.tile_pool(name="ehp", bufs=3) as eh_pool,
        tc.tile_pool(name="rcp", bufs=2) as rc_pool,
        tc.tile_pool(name="ps_sc", bufs=2, space="PSUM") as ps_sc,
        tc.tile_pool(name="ps_ac", bufs=1, space="PSUM") as ps_ac,
        tc.tile_pool(name="ps_rb", bufs=1, space="PSUM") as ps_rb,
        tc.tile_pool(name="ps_kb", bufs=2, space="PSUM") as ps_kb,
    ):
        for m in range(DC):
            # normalize this head-pair's K in place (k8 -> 16 * k-hat);
            # overlaps the previous pair's Act-bound exp work
            for b in range(NBLK):
                sl = slice(b * BLK, (b + 1) * BLK)
                kb = ps_kb.tile([P, BLK], F32, tag="kbc", name="kbps")
                mm(kb, selb_sb[:, m, :], reck[:, sl], start=True, stop=True)
                with nc.allow_low_precision(reason="fp8 pipeline"):
                    nc.vector.tensor_mul(k8[:, m, sl], k8[:, m, sl], kb)
            eh8 = eh_pool.tile([P, KC, 2, LQ], F8, tag="eh", name="eh8")
            for kc in range(KC):
                ps = ps_sc.tile([P, 2, 2 * LQ], F32, tag="sc", name="scps")
                for j in range(2):
                    mm(ps[:, j, 0:LQ],
                       k8[j * DH:(j + 1) * DH, m, kc * P:(kc + 1) * P],
                       q8[j * DH:(j + 1) * DH, m, :], start=True, stop=True)
                with nc.allow_low_precision(reason="fp8 pipeline"):
                    nc.scalar.activation(eh8[:, kc, :, :], ps[:, :, 0:LQ],
                                         func=AF.Exp, bias=0.0, scale=EXPS)
            for j in range(2):
                acc = ps_ac.tile([VW, LQ], F32, tag="ac", name="accps")
                for i in range(KC // 2):
                    mm(acc, v_sb[:, 2 * i:2 * i + 2, 2 * m + j, :],
                       eh8[:, 2 * i:2 * i + 2, j, :],
                       start=(i == 0), stop=(i == KC // 2 - 1), perf_mode=DR)
                recip = rc_pool.tile([1, LQ], F32R, tag="recip", name="recip")
                with nc.allow_low_precision(reason="coef"):
                    nc.vector.reciprocal(recip, acc[DH:DH + 1, :])
                rbc = ps_rb.tile([DH, LQ], F32, tag="rb", name="rbcps")
                mm(rbc, c64row[:, 0:DH], recip, start=True, stop=True)
                rbc_sb = rc_pool.tile([DH, LQ], F32, tag="rbcsb",
                                      name="rbcsb")
                nc.vector.tensor_copy(rbc_sb, rbc)
                with nc.allow_low_precision(reason="fp8 pipeline"):
                    nc.vector.tensor_mul(
                        attn8[j * DH:(j + 1) * DH, m, :], acc[0:DH, :],
                        rbc_sb)

    # out-projection + residual -> x2; LN2 -> normed2
    with (
        tc.tile_pool(name="upp", bufs=2) as up_pool,
        tc.tile_pool(name="lnc0", bufs=1) as lnc0,
        tc.tile_pool(name="ps_eo", bufs=2, space="PSUM") as ps_eo,
        tc.tile_pool(name="ps_l", bufs=1, space="PSUM") as ps_l,
        tc.tile_pool(name="ps_lb", bufs=1, space="PSUM") as ps_lb,
    ):
        for o in range(DC):
            ps = ps_eo.tile([P, LQ], F32, tag="ops", name="ops")
            for i in range(4):
                mm(ps, wo_sb[:, 2 * i:2 * i + 2, o * P:(o + 1) * P],
                   attn8[:, 2 * i:2 * i + 2, :], start=(i == 0),
                   stop=(i == 3), perf_mode=DR)
            upd = up_pool.tile([P, LQ], F32, tag="upd", name="upd")
            nc.scalar.activation(upd, ps, func=AF.Identity,
                                 bias=bo_sb[:, o:o + 1], scale=1.0 / (SA * SW))
            with nc.allow_low_precision(reason="f32r"):
                nc.vector.tensor_add(x2[:, o, :], upd, xo2[:, o, :])
        sums = ps_l.tile([1, LQ], F32, tag="lsum", name="lsums")
        sumsq = ps_l.tile([1, LQ], F32, tag="lsumsq", name="lsumsq")
        for c in range(DC):
            xsq = lnc0.tile([P, LQ], F32R, tag="lxsq", name="lxsq", bufs=2)
            nc.scalar.square(xsq, x2[:, c, :])
            mm(sums, ones_col, x2[:, c, :], start=(c == 0),
               stop=(c == DC - 1))
            mm(sumsq, ones_col, xsq, start=(c == 0), stop=(c == DC - 1))
        mu = lnc0.tile([1, LQ], F32, tag="lmu", name="lmu")
        nc.vector.tensor_scalar_mul(mu, sums, 1.0 / D)
        ex2 = lnc0.tile([1, LQ], F32, tag="lex2", name="lex2")
        nc.vector.tensor_scalar_mul(ex2, sumsq, 1.0 / D)
        var = lnc0.tile([1, LQ], F32, tag="lvar", name="lvar")
        nc.vector.tensor_mul(var, mu, mu)
        nc.vector.tensor_sub(var, ex2, var)
        sd = lnc0.tile([1, LQ], F32, tag="lsd", name="lsd")
        nc.scalar.activation(sd, var, func=AF.Sqrt, bias=eps_sb[0:1, :],
                             scale=1.0)
        coef = lnc0.tile([1, 2, LQ], F32R, tag="lcoef2", name="lcoef2")
        with nc.allow_low_precision(reason="coef"):
            nc.vector.reciprocal(coef[:, 0, :], sd)
            nc.vector.tensor_mul(coef[:, 1, :], mu, coef[:, 0, :])
            nc.vector.tensor_scalar_mul(coef[:, 1, :], coef[:, 1, :], -1.0)
        bc = ps_lb.tile([P, 2, LQ], F32, tag="lbc", name="lbc")
        mm(bc, ones_1x128, coef, start=True, stop=True)
        shift_sb = lnc0.tile([P, LQ], F32, tag="lshsb", name="lshsb")
        nc.scalar.copy(shift_sb, bc[:, 1, :])
        rb = bc[:, 0, :].unsqueeze(1).to_broadcast([P, DC, LQ])
        sb = shift_sb.unsqueeze(1).to_broadcast([P, DC, LQ])
        with nc.allow_low_precision(reason="bf16 pipeline"):
            nc.vector.tensor_mul(normed2, x2, rb)
            nc.gpsimd.tensor_add(normed2, normed2, sb)

    # FFN
    with (
        tc.tile_pool(name="osbp", bufs=2) as osb_pool,
        tc.tile_pool(name="ps_f1", bufs=3, space="PSUM") as ps_f1,
        tc.tile_pool(name="ps_f2", bufs=4, space="PSUM") as ps_f2,
    ):
        for f in range(FC):
            wf1m = wf1s.tile([P, DC, P], BF16, tag="wf1", name="wf1m")
            weng = nc.sync if f % 2 == 0 else nc.gpsimd
            weng.dma_start(wf1m, wf1h[:, f, :, :])
            ps = ps_f1.tile([P, LQ], F32, tag="f1", name="f1ps")
            for c in range(DC):
                mm(ps, wf1m[:, c, :], normed2[:, c, :], start=(c == 0),
                   stop=(c == DC - 1))
            with nc.allow_low_precision(reason="bf16 pipeline"):
                nc.scalar.activation(h_t[:, f, :], ps, func=GELU_FUNC,
                                     bias=bf1_sb[:, f:f + 1], scale=1.0)
        for g in range(2):
            accs = [ps_f2.tile([P, LQ], F32, tag="f2acc",
                               name=f"f2acc{i}") for i in range(4)]
            for f in range(FC):
                wf2m = wf2s.tile([P, 4, P], BF16, tag="wf2", name="wf2m")
                weng2 = nc.gpsimd if f % 2 == 0 else nc.sync
                weng2.dma_start(wf2m, wf2h[:, f, g * 4:(g + 1) * 4, :])
                for i in range(4):
                    mm(accs[i], wf2m[:, i, :], h_t[:, f, :],
                       start=(f == 0), stop=(f == FC - 1))
            for i in range(4):
                o = g * 4 + i
                osb = osb_pool.tile([P, LQ], F32, tag="osb", name="osb")
                nc.scalar.activation(osb, accs[i], func=AF.Identity,
                                     bias=bf2_sb[:, o:o + 1], scale=1.0)
                nc.vector.tensor_add(osb, osb, x2[:, o, :])
                nc.sync.dma_start(out3[:, o, :], osb)

    dep.release()
    ffp.release()
    midp.release()
    wf2s.release()
    wf1s.release()
    outp.release()
    scr_pool.release()
    singles.release()


_CACHED = None


def build():
    global _CACHED
    if _CACHED is None:
        nc = bacc.Bacc("TRN2", target_bir_lowering=False, debug=False)
        with tile.TileContext(nc) as tc:
            emit(tc)
        nc.compile()
        _CACHED = nc
    return _CACHED


def _selr_matrix():
    # [P, DC*H]: selr[p, m*16+h] = 1 iff h == 2m + (p >= 64)
    s = np.zeros((P, DC, H), np.float32)
    for m in range(DC):
        s[0:DH, m, 2 * m] = 1.0
        s[DH:P, m, 2 * m + 1] = 1.0
    return np.ascontiguousarray(s.reshape(P, P))


def _selb_matrix():
    # [H, DC*P]: selb[h, m*128+p] = 1 iff h == 2m + (p >= 64)
    s = np.zeros((H, DC, P), np.float32)
    for m in range(DC):
        s[2 * m, m, 0:DH] = 1.0
        s[2 * m + 1, m, DH:P] = 1.0
    return np.ascontiguousarray(s.reshape(H, DC * P))


def _chunk_pd(w):
    """[D, N] -> [128, D//128, N] with (p, c, n) = w[c*128+p, n]."""
    Dd, N = w.shape
    return np.ascontiguousarray(w.reshape(Dd // P, P, N).transpose(1, 0, 2))


def prep_inputs(inputs):
    """Host-side preprocessing: transpose x, scale/convert weights to fp8/bf16,
    fold LN gains/biases, precompute correction rows."""
    f = np.float32
    x = np.asarray(inputs["x"], f)
    lcc = np.asarray(inputs["lcc_values"], f)
    w_qkv = np.asarray(inputs["w_qkv"], f)
    b_qkv = np.asarray(inputs["b_qkv"], f)
    ln1_g = np.asarray(inputs["ln1_g"], f)
    ln1_b = np.asarray(inputs["ln1_b"], f)
    ln2_g = np.asarray(inputs["ln2_g"], f)
    ln2_b = np.asarray(inputs["ln2_b"], f)
    w_ff1 = np.asarray(inputs["w_ff1"], f)
    b_ff1 = np.asarray(inputs["b_ff1"], f)

    def chunked(b):  # [D] -> [128, DC] with chunk c in column c
        return np.ascontiguousarray(b.reshape(-1, P).T)

    wq = ln1_g[:, None] * w_qkv[:, 0:D]
    wk = ln1_g[:, None] * w_qkv[:, D:2 * D]
    wv = ln1_g[:, None] * w_qkv[:, 2 * D:3 * D]
    bq = b_qkv[0:D] + ln1_b @ w_qkv[:, 0:D]
    bk = b_qkv[D:2 * D] + ln1_b @ w_qkv[:, D:2 * D]
    bv = b_qkv[2 * D:3 * D] + ln1_b @ w_qkv[:, 2 * D:3 * D]
    wo = np.asarray(inputs["w_out"], f)
    wf1 = ln2_g[:, None] * w_ff1
    bf1f = b_ff1 + ln2_b @ w_ff1
    wf2 = np.asarray(inputs["w_ff2"], f)

    def cor_rows(w, b):
        # correction DR row: tile0 = -colsum(w)*(PS/SMU) paired with mu8,
        #                    tile1 = b*(PS/SSD) paired with sd8
        r = np.zeros((1, 2, D), f)
        r[0, 0] = -w.sum(axis=0) * (PS / SMU)
        r[0, 1] = b * (PS / SSD)
        return r.astype(NP_F8)

    xt = np.ascontiguousarray(x.T)

    # FFN weights pre-tiled for contiguous DMA: [128, FC, DC, 128]
    wf1t = np.ascontiguousarray(
        wf1.reshape(DC, P, FC, P).transpose(1, 2, 0, 3)).astype(NP_BF16)
    wf2t = np.ascontiguousarray(
        wf2.reshape(FC, P, DC, P).transpose(1, 0, 2, 3)).astype(NP_BF16)

    shared = {
        "xt": xt.astype(NP_BF16),
        "wq8": _chunk_pd(wq * SW).astype(NP_F8),
        "wk8": _chunk_pd(wk * SW).astype(NP_F8),
        "wv8": _chunk_pd(wv * SW).astype(NP_F8),
        "wo8": _chunk_pd(wo * SW).astype(NP_F8),
        "corq": cor_rows(wq, bq),
        "cork": cor_rows(wk, bk),
        "corv": cor_rows(wv, bv),
        "wf1h": wf1t,
        "wf2h": wf2t,
        "bo": chunked(np.asarray(inputs["b_out"], f)),
        "bf1": chunked(bf1f),
        "bf2": chunked(np.asarray(inputs["b_ff2"], f)),
        "lccel": np.ascontiguousarray(
            np.exp((lcc * (0.5 * LCC)).reshape(KC, P).T)),
        "selr": _selr_matrix().astype(NP_F8),
        "selrb": _selr_matrix().astype(NP_BF16),
        "selb": _selb_matrix(),
        "onesbd": np.ones((P, 1), NP_BF16),
        "ones1r": np.ones((1, P), np.float32),
        "c64r": np.full((1, P), SA * SSD / SKV, np.float32),
        "onescl": np.ones((P, 1), np.float32),
    }
    in_maps = []
    for c in range(NCORES):
        m = dict(shared)
        m["xot"] = np.ascontiguousarray(xt[:, c * LQ:(c + 1) * LQ])
        m["xotb"] = m["xot"].astype(NP_BF16)
        in_maps.append(m)
    return in_maps


def kernel(**inputs):
    nc = build()
    in_maps = prep_inputs(inputs)
    res = run_bass_kernel_spmd(nc, in_maps, core_ids=list(range(NCORES)))
    out = np.concatenate([res.results[c]["out_t"] for c in range(NCORES)], axis=1)
    return np.ascontiguousarray(out.T).astype(np.float32)



# revision 31
# speedup vs baseline: 51.7392x; 51.7392x over previous
"""Trainium2 Bass kernel for EnhancedMultiHeadSelfAttention (dense transformer block).

Sharding: sequence-parallel over 8 cores. Each core owns L/8 = 256 query rows.
K/V for all 2048 tokens are computed on every core from fp8 x; everything else
(Q, scores, softmax, attn@V, out-proj, LN2, FFN) is own-rows only. No
collectives.

Key structure (all activations feature-major [feature, token]):
 - LN1 is never applied to x. Projections run on raw fp8 x with an extra
   K=1 DoubleRow "correction row": out += (-colsum(W))*mu_t + b*sigma_t.
   The per-token rstd then cancels in cosine normalization for Q/K; for V,
   both rstd and the (key-side) lcc softmax bias are folded into a per-key
   scale applied during V's PSUM->SBUF copy, with the softmax denominator
   coming from an extra V column holding sigma*rstd*e^lcc = e^lcc terms.
 - K is cosine-normalized in place (fp8) so the exp over scores has a
   CONSTANT scale and no bias: one Activation op covers both heads of a
   pair (512 elements), halving Act-engine overhead on the exp path.
 - Q/K/V/out-proj and attn@V run as fp8e4 DoubleRow matmuls (two 128-row
   K-tiles per instruction at 0.5 cycles/row). FFN runs in bf16 (fp8 would
   exceed the 2e-2 error budget through the 4096-wide contraction).
 - clip(scores,-10,10) never binds and softmax needs no max-subtraction.

Scale conventions (stored value = scale * true value):
  x8 = 16 x     w{q,k,v,o}8 = 64 w    proj PSUM = 1024 * true
  k8 = 4 k~ then (after normalize) 16 k-hat      q8 = 16 q-hat
  v8 = 4 e^b v~ (b = key lcc bias + ln rstd)     mu8 = 256 mu
  sd8 = 16 sigma    den col = 16 e^b sigma       attn8 = 16 attn
"""

import numpy as np
import ml_dtypes

import concourse.bass as bass
import concourse.tile as tile
from concourse import bacc, mybir
from concourse.bass_utils import run_bass_kernel_spmd

F32 = mybir.dt.float32
F32R = mybir.dt.float32r
BF16 = mybir.dt.bfloat16
F8 = mybir.dt.float8e4
NP_F8 = ml_dtypes.float8_e4m3
NP_BF16 = ml_dtypes.bfloat16

L = 2048          # sequence length
D = 1024          # model dim
H = 16            # heads
DH = 64           # head dim
FF = 4096         # ffn hidden
P = 128           # partitions
NCORES = 8
LQ = L // NCORES  # 256 own query rows per core
DC = D // P       # 8 d-model chunks
FC = FF // P      # 32 ffn chunks
KC = L // P       # 16 key chunks
NBLK = 4          # token blocks of 512
BLK = L // NBLK   # 512

# CoreSim doesn't implement Gelu; test_sim swaps this to Identity and checks
# against a gelu-less reference. Hardware always uses the real (erf) Gelu.
GELU_FUNC = mybir.ActivationFunctionType.Gelu

LN_EPS = 1e-5
SCALING = DH ** -0.5
LCC = 0.1
DR = mybir.MatmulPerfMode.DoubleRow

# Quadratic softmax: exp(s) ~= (QA*s + QB)^2 + QD on s in [-0.125, 0.125]
QA = 0.7065543086717869
QB = 0.7087621823020126
QD = 0.4976622534108315
SEH = 256.0
RSEH = SEH ** 0.5
N_ACT = 12

SX = 16.0     # x8 scale
SW = 64.0     # fp8 weight scale
PS = SX * SW  # = 1024, scale of projection PSUM results
SKV = 4.0     # k~/v~ scale
SK8 = 16.0    # normalized k-hat scale
SQ8 = 16.0    # q8 scale (unit vectors * 16)
SMU = 256.0   # mu8 scale
SSD = 16.0    # sd8 (sigma) scale
SA = 16.0     # attn8 scale


def emit(tc):
    nc = tc.nc
    AF = mybir.ActivationFunctionType

    xt = nc.dram_tensor("xt", [D, L], BF16, kind="ExternalInput").ap()
    xot = nc.dram_tensor("xot", [D, LQ], F32, kind="ExternalInput").ap()
    xotb = nc.dram_tensor("xotb", [D, LQ], BF16, kind="ExternalInput").ap()
    wq8 = nc.dram_tensor("wq8", [P, DC, D], F8, kind="ExternalInput").ap()
    wk8 = nc.dram_tensor("wk8", [P, DC, D], F8, kind="ExternalInput").ap()
    wv8 = nc.dram_tensor("wv8", [P, DC, D], F8, kind="ExternalInput").ap()
    wo8 = nc.dram_tensor("wo8", [P, DC, D], F8, kind="ExternalInput").ap()
    corq = nc.dram_tensor("corq", [1, 2, D], F8, kind="ExternalInput").ap()
    cork = nc.dram_tensor("cork", [1, 2, D], F8, kind="ExternalInput").ap()
    corv = nc.dram_tensor("corv", [1, 2, D], F8, kind="ExternalInput").ap()
    wf1h = nc.dram_tensor("wf1h", [P, FC, DC, P], BF16, kind="ExternalInput").ap()
    wf2h = nc.dram_tensor("wf2h", [P, FC, DC, P], BF16, kind="ExternalInput").ap()
    bo = nc.dram_tensor("bo", [P, DC], F32, kind="ExternalInput").ap()
    bf1 = nc.dram_tensor("bf1", [P, FC], F32, kind="ExternalInput").ap()
    bf2 = nc.dram_tensor("bf2", [P, DC], F32, kind="ExternalInput").ap()
    lccel = nc.dram_tensor("lccel", [P, KC], F32, kind="ExternalInput").ap()
    selr = nc.dram_tensor("selr", [P, P], F8, kind="ExternalInput").ap()
    selrb = nc.dram_tensor("selrb", [P, P], BF16, kind="ExternalInput").ap()
    selb = nc.dram_tensor("selb", [H, DC * P], F32R, kind="ExternalInput").ap()
    onesbd = nc.dram_tensor("onesbd", [P, 1], BF16, kind="ExternalInput").ap()
    ones256d = nc.dram_tensor("ones256d", [1, 2 * P], BF16,
                              kind="ExternalInput").ap()
    ones8dd = nc.dram_tensor("ones8dd", [P, 2, 16], F8,
                             kind="ExternalInput").ap()
    ones1r = nc.dram_tensor("ones1r", [1, P], F32R, kind="ExternalInput").ap()
    c64r = nc.dram_tensor("c64r", [1, P], F32R, kind="ExternalInput").ap()
    onescl = nc.dram_tensor("onescl", [P, 1], F32R, kind="ExternalInput").ap()
    out_t = nc.dram_tensor("out_t", [D, LQ], F32, kind="ExternalOutput").ap()

    xt3 = xt.rearrange("(c p) t -> p c t", p=P)        # [128, 8, 2048]
    xot3 = xot.rearrange("(c p) t -> p c t", p=P)      # [128, 8, 256]
    out3 = out_t.rearrange("(c p) t -> p c t", p=P)    # [128, 8, 256]

    mm = nc.tensor.matmul

    # ---- persistent small constants -------------------------------------
    singles = tc.alloc_tile_pool(name="singles", bufs=1)
    ones_1x128 = singles.tile([1, P], F32R)  # K=1 broadcast lhsT (value 1)
    nc.scalar.dma_start(ones_1x128, ones1r)
    c64row = singles.tile([1, P], F32R)      # K=1 broadcast lhsT (value 64)
    nc.scalar.dma_start(c64row, c64r)
    ones_col = singles.tile([P, 1], F32R)    # K=128 -> M=1 reduction lhsT
    nc.scalar.dma_start(ones_col, onescl)
    onesb = singles.tile([P, 1], BF16)       # bf16 reduction lhsT
    nc.scalar.dma_start(onesb, onesbd)
    ones256 = singles.tile([1, 2 * P], BF16)
    nc.scalar.dma_start(ones256, ones256d)
    ones8dr_t = singles.tile([P, 2, 16], F8)
    nc.scalar.dma_start(ones8dr_t, ones8dd)
    selr8 = singles.tile([P, DC, H], F8)
    nc.scalar.dma_start(selr8, selr.rearrange("p (m h) -> p m h", h=H))
    selrb_sb = singles.tile([P, DC, H], BF16)
    nc.scalar.dma_start(selrb_sb, selrb.rearrange("p (m h) -> p m h", h=H))
    selb_sb = singles.tile([H, DC, P], F32R)
    nc.scalar.dma_start(selb_sb, selb.rearrange("h (m p) -> h m p", p=P))
    corq_sb = singles.tile([1, 2, D], F8)
    nc.scalar.dma_start(corq_sb, corq)
    cork_sb = singles.tile([1, 2, D], F8)
    nc.scalar.dma_start(cork_sb, cork)
    corv_sb = singles.tile([1, 2, D], F8)
    nc.scalar.dma_start(corv_sb, corv)
    bo_sb = singles.tile([P, DC], F32)
    nc.scalar.dma_start(bo_sb, bo)
    bf1_sb = singles.tile([P, FC], F32)
    nc.scalar.dma_start(bf1_sb, bf1)
    bf2_sb = singles.tile([P, DC], F32)
    nc.scalar.dma_start(bf2_sb, bf2)
    lcce_sb = singles.tile([P, KC], F32)
    nc.scalar.dma_start(lcce_sb, lccel)
    eps_sb = singles.tile([P, 1], F32)
    nc.vector.memset(eps_sb, LN_EPS)
    tbi_sb = singles.tile([P, 1], F32)
    nc.vector.memset(tbi_sb, RSEH * QB)
    # fp8 full weights (4 x 8KB/partition)
    wq_sb = singles.tile([P, DC, D], F8)
    nc.sync.dma_start(wq_sb, wq8)
    wk_sb = singles.tile([P, DC, D], F8)
    nc.sync.dma_start(wk_sb, wk8)
    wv_sb = singles.tile([P, DC, D], F8)
    wo_sb = singles.tile([P, DC, D], F8)
    # per-token stat tensors (filled during phases A/B)
    musd8 = singles.tile([1, 2, L], F8)       # [mu8 ; sd8] rows
    vscale = singles.tile([P, KC], F32)       # (SKV/PS) * rstd * e^lcc
    col8 = singles.tile([P, KC], F8)          # SSD * rstd * e^lcc (den col)
    reck = singles.tile([H, L], F32R)         # 4 / |k~| rows
    stat_r = singles.tile([P, NBLK, 2, 4], F32)   # [p, b, (sum,sumsq), kc%4]
    musd_r = singles.tile([P, NBLK, 2, 4], F8)    # repartitioned mu8/sd8

    scr_pool = tc.alloc_tile_pool(name="scr", bufs=1, space="DRAM")
    scr_st = scr_pool.tile([NBLK, 2, BLK], F32)    # stats rows -> repart
    scr_ms = scr_pool.tile([NBLK, 2, BLK], F8)     # mu/sd repart -> rows

    # ---- persistent activation tensors ----------------------------------
    outp = tc.alloc_tile_pool(name="outp", bufs=1)
    x2 = outp.tile([P, DC, LQ], F32R)
    attn8 = outp.tile([P, DC, LQ], F8)
    wf1s = tc.alloc_tile_pool(name="wf1s", bufs=9)
    wf2s = tc.alloc_tile_pool(name="wf2s", bufs=6)
    midp = tc.alloc_tile_pool(name="midp", bufs=1)
    k8 = midp.tile([P, DC, L], F8)
    VW = 80  # 64 values + 1 denominator + 15 pad (dual-fp8 M%16==0)
    v_sb = midp.tile([P, KC, H, VW], F8)
    q8 = midp.tile([P, DC, LQ], F8)
    dsum = midp.tile([1, H, VW], BF16)
    x8p = tc.alloc_tile_pool(name="x8p", bufs=1)
    x8 = x8p.tile([P, DC, L], F8)

    # =====================================================================
    # Phase A: x -> fp8, per-token stats; Phase C: Q projection + normalize
    # (emitted together so the scheduler can overlap them)
    # =====================================================================
    with (
        tc.tile_pool(name="xblk", bufs=2) as xblk_pool,
        tc.tile_pool(name="xsqp", bufs=2) as xsq_pool,
        tc.tile_pool(name="stm", bufs=2) as stm_pool,
        tc.tile_pool(name="strow", bufs=1) as strow_pool,
        tc.tile_pool(name="qcp", bufs=1) as qc_pool,
        tc.tile_pool(name="qsqp", bufs=2) as qsq_pool,
        tc.tile_pool(name="ps_st", bufs=1, space="PSUM") as ps_st,
        tc.tile_pool(name="ps_q", bufs=2, space="PSUM") as ps_q,
        tc.tile_pool(name="ps_qn", bufs=1, space="PSUM") as ps_qn,
        tc.tile_pool(name="ps_qb", bufs=1, space="PSUM") as ps_qb,
    ):
        xdma = [nc.sync, nc.scalar, nc.gpsimd, nc.sync]
        xcvt = [("act", "dve"), ("pool", "act"), ("dve", "pool"),
                ("act", "dve")]
        for b in range(NBLK):
            sl = slice(b * BLK, (b + 1) * BLK)
            xblk = xblk_pool.tile([P, DC, BLK], BF16, tag="xblk")
            hb = BLK // 2
            xdma[b].dma_start(xblk[:, :, 0:hb], xt3[:, :, sl][:, :, 0:hb])
            xdma[b].dma_start(xblk[:, :, hb:BLK], xt3[:, :, sl][:, :, hb:BLK])
            if b == NBLK - 1:
                # V/out-proj weights load after all x blocks are queued
                nc.gpsimd.dma_start(wv_sb, wv8)
                nc.gpsimd.dma_start(wo_sb, wo8)
            with nc.allow_low_precision(reason="fp8 pipeline"):
                for h, eng in enumerate(xcvt[b]):
                    hs = slice(b * BLK + h * hb, b * BLK + (h + 1) * hb)
                    hx = slice(h * hb, (h + 1) * hb)
                    if eng == "act":
                        nc.scalar.activation(x8[:, :, hs], xblk[:, :, hx],
                                             func=AF.Copy, bias=0.0, scale=SX)
                    elif eng == "dve":
                        nc.vector.tensor_scalar_mul(x8[:, :, hs],
                                                    xblk[:, :, hx], SX)
                    else:
                        nc.gpsimd.tensor_scalar_mul(x8[:, :, hs],
                                                    xblk[:, :, hx], SX)
                xsqb = xsq_pool.tile([P, DC, BLK], BF16, tag="xsq")
                nc.vector.tensor_mul(xsqb, xblk, xblk)
            sums = ps_st.tile([16, BLK], F32, tag="sums")
            sumsq = ps_st.tile([1, BLK], F32, tag="sumsq")
            for i in range(4):
                mm(sums, ones8dr_t, x8[:, 2 * i:2 * i + 2, sl],
                   start=(i == 0), stop=(i == 3), perf_mode=DR)
            for c in range(DC):
                mm(sumsq, onesb, xsqb[:, c, :], start=(c == 0),
                   stop=(c == DC - 1))
            # stage stat rows to SBUF, roundtrip via DRAM to [128, ...] layout
            statrow = strow_pool.tile([1, 2, BLK], F32, tag="strow")
            nc.vector.tensor_copy(statrow[:, 0, :], sums[0:1, :])
            nc.vector.tensor_copy(statrow[:, 1, :], sumsq)
            nc.sync.dma_start(scr_st[b:b + 1], statrow)
            nc.sync.dma_start(
                stat_r[:, b, :, :],
                scr_st[b].rearrange("j (q p) -> p j q", p=P))
            # per-token coefficient math in [128, 4] layout
            mu = stm_pool.tile([P, 4], F32, tag="mu")
            nc.vector.tensor_scalar_mul(mu, stat_r[:, b, 0, :], 1.0 / (D * SX))
            ex2 = stm_pool.tile([P, 4], F32, tag="ex2")
            nc.vector.tensor_scalar_mul(ex2, stat_r[:, b, 1, :], 1.0 / D)
            var = stm_pool.tile([P, 4], F32, tag="var")
            nc.vector.tensor_mul(var, mu, mu)
            nc.vector.tensor_sub(var, ex2, var)
            sd = stm_pool.tile([P, 4], F32, tag="sd")
            nc.scalar.activation(sd, var, func=AF.Sqrt, bias=eps_sb, scale=1.0)
            rstd = stm_pool.tile([P, 4], F32, tag="rstd")
            with nc.allow_low_precision(reason="coef"):
                nc.vector.reciprocal(rstd, sd)
            kcs = slice(b * 4, (b + 1) * 4)
            relcc = stm_pool.tile([P, 4], F32, tag="relcc")
            nc.vector.tensor_mul(relcc, rstd, lcce_sb[:, kcs])
            nc.vector.tensor_scalar_mul(vscale[:, kcs], relcc, SKV / PS)
            with nc.allow_low_precision(reason="fp8 pipeline"):
                nc.vector.tensor_scalar_mul(col8[:, kcs], relcc, SSD)
                nc.vector.tensor_scalar_mul(musd_r[:, b, 0, :], mu, SMU)
                nc.vector.tensor_scalar_mul(musd_r[:, b, 1, :], sd, SSD)
            nc.sync.dma_start(
                scr_ms[b].rearrange("j (q p) -> p j q", p=P),
                musd_r[:, b, :, :])
            for j in range(2):
                nc.sync.dma_start(
                    musd8[:, j, sl],
                    scr_ms[b, j].rearrange("(o t) -> o t", o=1))

        # ---- Phase C: Q (own tokens; stats recomputed from xot since the
        # shared program can't address its own slice of musd8) --------------
        xo_blk = qc_pool.tile([P, DC, LQ], BF16, name="xo_blk")
        nc.sync.dma_start(xo_blk, xotb.rearrange("(c p) t -> p c t", p=P))
        x8own = qc_pool.tile([P, DC, LQ], F8, name="x8own")
        with nc.allow_low_precision(reason="fp8 pipeline"):
            nc.scalar.activation(x8own, xo_blk, func=AF.Copy, bias=0.0,
                                 scale=SX)
        ps_os = ps_qn.tile([1, LQ], F32, tag="osum")
        for c in range(DC):
            mm(ps_os, onesb, xo_blk[:, c, :], start=(c == 0),
               stop=(c == DC - 1))
        osr = qc_pool.tile([1, 2, LQ], F32, name="osr")
        nc.vector.tensor_copy(osr[:, 0, :], ps_os)
        xsq_o = qc_pool.tile([P, DC, LQ], BF16, name="xsq_o")
        with nc.allow_low_precision(reason="bf16 pipeline"):
            nc.vector.tensor_mul(xsq_o, xo_blk, xo_blk)
        for c in range(DC):
            mm(ps_os, onesb, xsq_o[:, c, :], start=(c == 0),
               stop=(c == DC - 1))
        nc.vector.tensor_copy(osr[:, 1, :], ps_os)
        mu_o = qc_pool.tile([1, LQ], F32, name="mu_o")
        nc.vector.tensor_scalar_mul(mu_o, osr[:, 0, :], 1.0 / D)
        ex2_o = qc_pool.tile([1, LQ], F32, name="ex2_o")
        nc.vector.tensor_scalar_mul(ex2_o, osr[:, 1, :], 1.0 / D)
        var_o = qc_pool.tile([1, LQ], F32, name="var_o")
        nc.vector.tensor_mul(var_o, mu_o, mu_o)
        nc.vector.tensor_sub(var_o, ex2_o, var_o)
        sd_o = qc_pool.tile([1, LQ], F32, name="sd_o")
        nc.scalar.activation(sd_o, var_o, func=AF.Sqrt, bias=eps_sb[0:1, :],
                             scale=1.0)
        musd_own = qc_pool.tile([1, 2, LQ], F8, name="musd_own")
        with nc.allow_low_precision(reason="fp8 pipeline"):
            nc.vector.tensor_scalar_mul(musd_own[:, 0, :], mu_o, SMU)
            nc.vector.tensor_scalar_mul(musd_own[:, 1, :], sd_o, SSD)
        # Q DoubleRow projections + per-head cosine normalization
        qt_sb = qc_pool.tile([P, DC, LQ], BF16, name="qt_sb")
        nsq_q = ps_qn.tile([H, LQ], F32, tag="qn")
        qsq8 = None
        for m in range(DC):
            ps = ps_q.tile([P, LQ], F32, tag="qps")
            for i in range(4):
                mm(ps, wq_sb[:, 2 * i:2 * i + 2, m * P:(m + 1) * P],
                   x8own[:, 2 * i:2 * i + 2, :], start=(i == 0), stop=False,
                   perf_mode=DR)
            mm(ps, corq_sb[:, :, m * P:(m + 1) * P], musd_own,
               start=False, stop=True, perf_mode=DR)
            with nc.allow_low_precision(reason="bf16 pipeline"):
                nc.vector.tensor_scalar_mul(qt_sb[:, m, :], ps, SKV / PS)
            if m % 2 == 0:
                qsq8 = qsq_pool.tile([P, 2, LQ], F8, tag="qsq")
            nc.scalar.activation(qsq8[:, m % 2, :], ps, func=AF.Square,
                                 bias=0.0, scale=2.0 / PS)
            if m % 2 == 1:
                mm(nsq_q, selr8[:, m - 1:m + 1, :], qsq8,
                   start=(m == 1), stop=(m == DC - 1), perf_mode=DR)
        qs = qc_pool.tile([H, LQ], F32, name="qs")
        nc.scalar.activation(qs, nsq_q, func=AF.Sqrt, bias=0.0, scale=1.0)
        nc.vector.tensor_scalar_max(qs, qs, 1e-8)
        rec = qc_pool.tile([H, LQ], F32R, name="qrec")
        with nc.allow_low_precision(reason="coef"):
            nc.vector.reciprocal(rec, qs)
        nc.vector.tensor_scalar_mul(rec, rec, 2.0 * SQ8 / SKV)
        for m in range(DC):
            bc = ps_qb.tile([P, LQ], F32, tag="qbc")
            mm(bc, selb_sb[:, m, :], rec, start=True, stop=True)
            with nc.allow_low_precision(reason="fp8 pipeline"):
                nc.vector.tensor_mul(q8[:, m, :], qt_sb[:, m, :], bc)

    # sigma/e^lcc column of V (denominator source) + zero pad columns
    with nc.allow_low_precision(reason="fp8 pipeline"):
        nc.vector.memset(v_sb[:, :, :, DH + 1:VW], 0.0)
        nc.vector.tensor_copy(
            v_sb[:, :, :, DH], col8.unsqueeze(2).to_broadcast([P, KC, H]))

    # =====================================================================
    # Phase B: K and V projections (fp8 DR); K cosine-normalized in place
    # =====================================================================
    with (
        tc.tile_pool(name="ksqp", bufs=2) as ksq_pool,
        tc.tile_pool(name="nsqs", bufs=2) as nsqs_pool,
        tc.tile_pool(name="ps_k", bufs=2, space="PSUM") as ps_k,
        tc.tile_pool(name="ps_v", bufs=2, space="PSUM") as ps_v,
        tc.tile_pool(name="ps_n", bufs=2, space="PSUM") as ps_n,
    ):
        for b in range(NBLK):
            sl = slice(b * BLK, (b + 1) * BLK)
            ms = musd8[:, :, sl]
            for m in range(DC):
                ps = ps_k.tile([P, BLK], F32, tag="kps")
                for i in range(4):
                    mm(ps, wk_sb[:, 2 * i:2 * i + 2, m * P:(m + 1) * P],
                       x8[:, 2 * i:2 * i + 2, sl], start=(i == 0), stop=False,
                       perf_mode=DR)
                mm(ps, cork_sb[:, :, m * P:(m + 1) * P], ms,
                   start=False, stop=True, perf_mode=DR)
                with nc.allow_low_precision(reason="fp8 pipeline"):
                    if m % 2 == 0:
                        nc.vector.tensor_scalar_mul(k8[:, m, sl], ps, SKV / PS)
                    else:
                        nc.scalar.activation(k8[:, m, sl], ps, func=AF.Copy,
                                             bias=0.0, scale=SKV / PS)
            # V for this block's 4 token chunks (scaled per key by vscale)
            for t in range(b * 4, (b + 1) * 4):
                tsl = slice(t * P, (t + 1) * P)
                for g in range(2):
                    csl = slice(g * BLK, (g + 1) * BLK)
                    ps = ps_v.tile([P, BLK], F32, tag="vps")
                    for i in range(4):
                        mm(ps, x8[:, 2 * i:2 * i + 2, tsl],
                           wv_sb[:, 2 * i:2 * i + 2, csl],
                           start=(i == 0), stop=False, perf_mode=DR)
                    mm(ps, musd8[:, :, tsl], corv_sb[:, :, csl],
                       start=False, stop=True, perf_mode=DR)
                    ps_h = ps.rearrange("p (h d) -> p h d", d=DH)
                    dst = v_sb[:, t, g * DC:(g + 1) * DC, 0:DH]
                    with nc.allow_low_precision(reason="fp8 pipeline"):
                        if g == 0:
                            nc.vector.tensor_scalar_mul(
                                dst, ps_h, vscale[:, t:t + 1])
                        else:
                            nc.scalar.activation(dst, ps_h, func=AF.Copy,
                                                 bias=0.0,
                                                 scale=vscale[:, t:t + 1])
            # k norms: squares on Pool (fp8), per-head sums via DR, then reck
            nsq = ps_n.tile([H, BLK], F32, tag="nsq")
            for i in range(4):
                ksq8 = ksq_pool.tile([P, 2, BLK], F8, tag="ksq")
                with nc.allow_low_precision(reason="fp8 pipeline"):
                    for j in range(2):
                        m = 2 * i + j
                        nc.gpsimd.tensor_mul(ksq8[:, j, :], k8[:, m, sl],
                                             k8[:, m, sl])
                mm(nsq, selr8[:, 2 * i:2 * i + 2, :], ksq8, start=(i == 0),
                   stop=(i == 3), perf_mode=DR)
            nsq_sb = nsqs_pool.tile([H, BLK], F32, tag="nsqs")
            nc.scalar.activation(nsq_sb, nsq, func=AF.Sqrt, bias=0.0,
                                 scale=1.0)
            nc.vector.tensor_scalar_max(nsq_sb, nsq_sb, 1e-8)
            with nc.allow_low_precision(reason="coef"):
                nc.vector.reciprocal(reck[:, sl], nsq_sb)
            nc.vector.tensor_scalar_mul(reck[:, sl], reck[:, sl], SK8)

    x8p.release()

    # column sums of v_sb scaled by SEH*QD (the "+QD" softmax-weight term)
    with tc.tile_pool(name="ps_cs", bufs=3, space="PSUM") as ps_cs:
        for h0, hn in ((0, 6), (6, 6), (12, 4)):
            cs = ps_cs.tile([16, hn * VW], F32, tag="cs", name="csps")
            for i in range(KC // 2):
                mm(cs, ones8dr_t,
                   v_sb[:, 2 * i:2 * i + 2, h0:h0 + hn, :].rearrange(
                       "p a h w -> p a (h w)"),
                   start=(i == 0), stop=(i == KC // 2 - 1), perf_mode=DR)
            with nc.allow_low_precision(reason="bf16"):
                nc.vector.tensor_scalar_mul(
                    dsum[:, h0:h0 + hn, :].rearrange("o h w -> o (h w)"),
                    cs[0:1, :], SEH * QD)

    # =====================================================================
    # Phases D/E/F (full query width): scores -> quad weights -> attn@V ->
    # out-proj -> LN2 -> FFN
    # =====================================================================
    EXPS = SCALING / (SQ8 * SK8)
    TSC = RSEH * QA * EXPS
    TBI = RSEH * QB

    ffp = tc.alloc_tile_pool(name="ffp", bufs=1)
    h_t = ffp.tile([P, FC, LQ], BF16)
    normed2 = ffp.tile([P, DC, LQ], BF16)
    dep = tc.alloc_tile_pool(name="dep", bufs=1)
    xo2 = dep.tile([P, DC, LQ], F32)
    nc.sync.dma_start(xo2, xot3)

    with (
        tc.tile_pool(name="ehp", bufs=3) as eh_pool,
        tc.tile_pool(name="tqp", bufs=6) as tq_pool,
        tc.tile_pool(name="rcp", bufs=2) as rc_pool,
        tc.tile_pool(name="ps_sc", bufs=2, space="PSUM") as ps_sc,
        tc.tile_pool(name="ps_ac", bufs=2, space="PSUM") as ps_ac,
        tc.tile_pool(name="ps_rb", bufs=1, space="PSUM") as ps_rb,
        tc.tile_pool(name="ps_kb", bufs=1, space="PSUM") as ps_kb,
    ):
        for m in range(DC):
            # normalize this head-pair's K in place (k8 -> 16 * k-hat);
            # overlaps the previous pair's Act-bound exp work
            for b in range(NBLK):
                sl = slice(b * BLK, (b + 1) * BLK)
                kb = ps_kb.tile([P, BLK], F32, tag="kbc", name="kbps")
                mm(kb, selb_sb[:, m, :], reck[:, sl], start=True, stop=True)
                with nc.allow_low_precision(reason="fp8 pipeline"):
                    nc.vector.tensor_mul(k8[:, m, sl], k8[:, m, sl], kb)
            eh8 = eh_pool.tile([P, KC, 2, LQ], F8, tag="eh", name="eh8")
            for kc in range(KC):
                ps = ps_sc.tile([P, 2, 2 * LQ], F32, tag="sc", name="scps")
                for j in range(2):
                    mm(ps[:, j, 0:LQ],
                       k8[j * DH:(j + 1) * DH, m, kc * P:(kc + 1) * P],
                       q8[j * DH:(j + 1) * DH, m, :], start=True, stop=True)
                # NOTE: merging two kc per PSUM tile ([P,2,2,LQ]) passes
                # CoreSim but dies at runtime on HW -- keep per-kc tiles.
                with nc.allow_low_precision(reason="fp8 pipeline"):
                    if kc < N_ACT:
                        nc.scalar.activation(eh8[:, kc, :, :], ps[:, :, 0:LQ],
                                             func=AF.Square, bias=tbi_sb,
                                             scale=TSC)
                    else:
                        tq = tq_pool.tile([P, 2, LQ], BF16, tag="tq",
                                          name="tq")
                        nc.vector.tensor_scalar(
                            tq, ps[:, :, 0:LQ], TSC, TBI,
                            mybir.AluOpType.mult, mybir.AluOpType.add)
                        nc.gpsimd.tensor_mul(eh8[:, kc, :, :], tq, tq)
            for j in range(2):
                acc = ps_ac.tile([VW, LQ], F32, tag="ac", name="accps")
                for i in range(KC // 2):
                    mm(acc, v_sb[:, 2 * i:2 * i + 2, 2 * m + j, :],
                       eh8[:, 2 * i:2 * i + 2, j, :],
                       start=(i == 0), stop=False, perf_mode=DR)
                mm(acc, dsum[:, 2 * m + j, :], ones256,
                   start=False, stop=True)
                recip = rc_pool.tile([1, LQ], F32R, tag="recip", name="recip")
                with nc.allow_low_precision(reason="coef"):
                    nc.vector.reciprocal(recip, acc[DH:DH + 1, :])
                rbc = ps_rb.tile([DH, LQ], F32, tag="rb", name="rbcps")
                mm(rbc, c64row[:, 0:DH], recip, start=True, stop=True)
                rbc_sb = rc_pool.tile([DH, LQ], F32, tag="rbcsb",
                                      name="rbcsb")
                nc.scalar.copy(rbc_sb, rbc)
                with nc.allow_low_precision(reason="fp8 pipeline"):
                    nc.vector.tensor_mul(
                        attn8[j * DH:(j + 1) * DH, m, :], acc[0:DH, :],
                        rbc_sb)

    # out-projection + residual -> x2; LN2 -> normed2
    with (
        tc.tile_pool(name="upp", bufs=2) as up_pool,
        tc.tile_pool(name="lnc0", bufs=1) as lnc0,
        tc.tile_pool(name="ps_eo", bufs=2, space="PSUM") as ps_eo,
        tc.tile_pool(name="ps_l", bufs=1, space="PSUM") as ps_l,
        tc.tile_pool(name="ps_lb", bufs=1, space="PSUM") as ps_lb,
    ):
        for o in range(DC):
            ps = ps_eo.tile([P, LQ], F32, tag="ops", name="ops")
            for i in range(4):
                mm(ps, wo_sb[:, 2 * i:2 * i + 2, o * P:(o + 1) * P],
                   attn8[:, 2 * i:2 * i + 2, :], start=(i == 0),
                   stop=(i == 3), perf_mode=DR)
            upd = up_pool.tile([P, LQ], F32, tag="upd", name="upd")
            nc.scalar.activation(upd, ps, func=AF.Identity,
                                 bias=bo_sb[:, o:o + 1], scale=1.0 / (SA * SW))
            with nc.allow_low_precision(reason="f32r"):
                nc.vector.tensor_add(x2[:, o, :], upd, xo2[:, o, :])
        sums = ps_l.tile([1, LQ], F32, tag="lsum", name="lsums")
        sumsq = ps_l.tile([1, LQ], F32, tag="lsumsq", name="lsumsq")
        for c in range(DC):
            xsq = lnc0.tile([P, LQ], F32R, tag="lxsq", name="lxsq", bufs=2)
            nc.scalar.square(xsq, x2[:, c, :])
            mm(sums, ones_col, x2[:, c, :], start=(c == 0),
               stop=(c == DC - 1))
            mm(sumsq, ones_col, xsq, start=(c == 0), stop=(c == DC - 1))
        mu = lnc0.tile([1, LQ], F32, tag="lmu", name="lmu")
        nc.vector.tensor_scalar_mul(mu, sums, 1.0 / D)
        ex2 = lnc0.tile([1, LQ], F32, tag="lex2", name="lex2")
        nc.vector.tensor_scalar_mul(ex2, sumsq, 1.0 / D)
        var = lnc0.tile([1, LQ], F32, tag="lvar", name="lvar")
        nc.vector.tensor_mul(var, mu, mu)
        nc.vector.tensor_sub(var, ex2, var)
        sd = lnc0.tile([1, LQ], F32, tag="lsd", name="lsd")
        nc.scalar.activation(sd, var, func=AF.Sqrt, bias=eps_sb[0:1, :],
                             scale=1.0)
        coef = lnc0.tile([1, 2, LQ], F32R, tag="lcoef2", name="lcoef2")
        with nc.allow_low_precision(reason="coef"):
            nc.vector.reciprocal(coef[:, 0, :], sd)
            nc.vector.tensor_mul(coef[:, 1, :], mu, coef[:, 0, :])
            nc.vector.tensor_scalar_mul(coef[:, 1, :], coef[:, 1, :], -1.0)
        bc = ps_lb.tile([P, 2, LQ], F32, tag="lbc", name="lbc")
        mm(bc, ones_1x128, coef, start=True, stop=True)
        shift_sb = lnc0.tile([P, LQ], F32, tag="lshsb", name="lshsb")
        nc.scalar.copy(shift_sb, bc[:, 1, :])
        rb = bc[:, 0, :].unsqueeze(1).to_broadcast([P, DC, LQ])
        sb = shift_sb.unsqueeze(1).to_broadcast([P, DC, LQ])
        with nc.allow_low_precision(reason="bf16 pipeline"):
            nc.vector.tensor_mul(normed2, x2, rb)
            nc.gpsimd.tensor_add(normed2, normed2, sb)

    # FFN
    with (
        tc.tile_pool(name="osbp", bufs=2) as osb_pool,
        tc.tile_pool(name="ps_f1", bufs=3, space="PSUM") as ps_f1,
        tc.tile_pool(name="ps_f2", bufs=4, space="PSUM") as ps_f2,
    ):
        for f in range(FC):
            wf1m = wf1s.tile([P, DC, P], BF16, tag="wf1", name="wf1m")
            weng = nc.sync if f % 2 == 0 else nc.gpsimd
            weng.dma_start(wf1m, wf1h[:, f, :, :])
            ps = ps_f1.tile([P, LQ], F32, tag="f1", name="f1ps")
            for c in range(DC):
                mm(ps, wf1m[:, c, :], normed2[:, c, :], start=(c == 0),
                   stop=(c == DC - 1))
            with nc.allow_low_precision(reason="bf16 pipeline"):
                nc.scalar.activation(h_t[:, f, :], ps, func=GELU_FUNC,
                                     bias=bf1_sb[:, f:f + 1], scale=1.0)
        for g in range(2):
            accs = [ps_f2.tile([P, LQ], F32, tag="f2acc",
                               name=f"f2acc{i}") for i in range(4)]
            for f in range(FC):
                wf2m = wf2s.tile([P, 4, P], BF16, tag="wf2", name="wf2m")
                weng2 = nc.gpsimd if f % 2 == 0 else nc.sync
                weng2.dma_start(wf2m, wf2h[:, f, g * 4:(g + 1) * 4, :])
                for i in range(4):
                    mm(accs[i], wf2m[:, i, :], h_t[:, f, :],
                       start=(f == 0), stop=(f == FC - 1))
            for i in range(4):
                o = g * 4 + i
                osb = osb_pool.tile([P, LQ], F32, tag="osb", name="osb")
                nc.scalar.activation(osb, accs[i], func=AF.Identity,
                                     bias=bf2_sb[:, o:o + 1], scale=1.0)
                nc.vector.tensor_add(osb, osb, x2[:, o, :])
                nc.sync.dma_start(out3[:, o, :], osb)

    dep.release()
    ffp.release()
    midp.release()
    wf2s.release()
    wf1s.release()
    outp.release()
    scr_pool.release()
    singles.release()


_CACHED = None


def build():
    global _CACHED
    if _CACHED is None:
        nc = bacc.Bacc("TRN2", target_bir_lowering=False, debug=False)
        with tile.TileContext(nc) as tc:
            emit(tc)
        nc.compile()
        _CACHED = nc
    return _CACHED


def _selr_matrix():
    # [P, DC*H]: selr[p, m*16+h] = 1 iff h == 2m + (p >= 64)
    s = np.zeros((P, DC, H), np.float32)
    for m in range(DC):
        s[0:DH, m, 2 * m] = 1.0
        s[DH:P, m, 2 * m + 1] = 1.0
    return np.ascontiguousarray(s.reshape(P, P))


def _selb_matrix():
    # [H, DC*P]: selb[h, m*128+p] = 1 iff h == 2m + (p >= 64)
    s = np.zeros((H, DC, P), np.float32)
    for m in range(DC):
        s[2 * m, m, 0:DH] = 1.0
        s[2 * m + 1, m, DH:P] = 1.0
    return np.ascontiguousarray(s.reshape(H, DC * P))


def _chunk_pd(w):
    """[D, N] -> [128, D//128, N] with (p, c, n) = w[c*128+p, n]."""
    Dd, N = w.shape
    return np.ascontiguousarray(w.reshape(Dd // P, P, N).transpose(1, 0, 2))


def prep_inputs(inputs):
    """Host-side preprocessing: transpose x, scale/convert weights to fp8/bf16,
    fold LN gains/biases, precompute correction rows."""
    f = np.float32
    x = np.asarray(inputs["x"], f)
    lcc = np.asarray(inputs["lcc_values"], f)
    w_qkv = np.asarray(inputs["w_qkv"], f)
    b_qkv = np.asarray(inputs["b_qkv"], f)
    ln1_g = np.asarray(inputs["ln1_g"], f)
    ln1_b = np.asarray(inputs["ln1_b"], f)
    ln2_g = np.asarray(inputs["ln2_g"], f)
    ln2_b = np.asarray(inputs["ln2_b"], f)
    w_ff1 = np.asarray(inputs["w_ff1"], f)
    b_ff1 = np.asarray(inputs["b_ff1"], f)

    def chunked(b):  # [D] -> [128, DC] with chunk c in column c
        return np.ascontiguousarray(b.reshape(-1, P).T)

    wq = ln1_g[:, None] * w_qkv[:, 0:D]
    wk = ln1_g[:, None] * w_qkv[:, D:2 * D]
    wv = ln1_g[:, None] * w_qkv[:, 2 * D:3 * D]
    bq = b_qkv[0:D] + ln1_b @ w_qkv[:, 0:D]
    bk = b_qkv[D:2 * D] + ln1_b @ w_qkv[:, D:2 * D]
    bv = b_qkv[2 * D:3 * D] + ln1_b @ w_qkv[:, 2 * D:3 * D]
    wo = np.asarray(inputs["w_out"], f)
    wf1 = ln2_g[:, None] * w_ff1
    bf1f = b_ff1 + ln2_b @ w_ff1
    wf2 = np.asarray(inputs["w_ff2"], f)

    def cor_rows(w, b):
        # correction DR row: tile0 = -colsum(w)*(PS/SMU) paired with mu8,
        #                    tile1 = b*(PS/SSD) paired with sd8
        r = np.zeros((1, 2, D), f)
        r[0, 0] = -w.sum(axis=0) * (PS / SMU)
        r[0, 1] = b * (PS / SSD)
        return r.astype(NP_F8)

    xt = np.ascontiguousarray(x.T)

    # FFN weights pre-tiled for contiguous DMA: [128, FC, DC, 128]
    wf1t = np.ascontiguousarray(
        wf1.reshape(DC, P, FC, P).transpose(1, 2, 0, 3)).astype(NP_BF16)
    wf2t = np.ascontiguousarray(
        wf2.reshape(FC, P, DC, P).transpose(1, 0, 2, 3)).astype(NP_BF16)

    shared = {
        "xt": xt.astype(NP_BF16),
        "wq8": _chunk_pd(wq * SW).astype(NP_F8),
        "wk8": _chunk_pd(wk * SW).astype(NP_F8),
        "wv8": _chunk_pd(wv * SW).astype(NP_F8),
        "wo8": _chunk_pd(wo * SW).astype(NP_F8),
        "corq": cor_rows(wq, bq),
        "cork": cor_rows(wk, bk),
        "corv": cor_rows(wv, bv),
        "wf1h": wf1t,
        "wf2h": wf2t,
        "bo": chunked(np.asarray(inputs["b_out"], f)),
        "bf1": chunked(bf1f),
        "bf2": chunked(np.asarray(inputs["b_ff2"], f)),
        "lccel": np.ascontiguousarray(
            np.exp((lcc * (0.5 * LCC)).reshape(KC, P).T)),
        "selr": _selr_matrix().astype(NP_F8),
        "selrb": _selr_matrix().astype(NP_BF16),
        "selb": _selb_matrix(),
        "onesbd": np.ones((P, 1), NP_BF16),
        "ones256d": np.ones((1, 2 * P), NP_BF16),
        "ones8dd": np.ones((P, 2, 16), NP_F8),
        "ones1r": np.ones((1, P), np.float32),
        "c64r": np.full((1, P), SA * SSD / SKV, np.float32),
        "onescl": np.ones((P, 1), np.float32),
    }
    in_maps = []
    for c in range(NCORES):
        m = dict(shared)
        m["xot"] = np.ascontiguousarray(xt[:, c * LQ:(c + 1) * LQ])
        m["xotb"] = m["xot"].astype(NP_BF16)
        in_maps.append(m)
    return in_maps


def kernel(**inputs):
    nc = build()
    in_maps = prep_inputs(inputs)
    res = run_bass_kernel_spmd(nc, in_maps, core_ids=list(range(NCORES)))
    out = np.concatenate([res.results[c]["out_t"] for c in range(NCORES)], axis=1)
    return np.ascontiguousarray(out.T).astype(np.float32)

